# revision 7
# baseline (speedup 1.0000x reference)
"""Causal self-attention (B=4, T=2048, C=768, H=12) on 8 trn2 NeuronCores.

Sharding: core c -> batch c//2, head-group c%2 (6 heads each).
Each core computes qkv projection, flash-style causal attention and its
partial c_proj contribution for its 6 heads; the host sums the two
head-group partials per batch and adds b_proj.

Schedule notes:
- All DMAs share one HW ring (FIFO), so loads are emitted in first-use
  order and the boot qkv projection runs contraction-chunk-outer so the
  PE consumes input chunks as they arrive.
- Q^T/K^T live in per-(head-pair, 512-token-chunk) tiles so Tile's
  per-tile dependency tracking gates each attention unit only on the
  chunk it actually reads.  S matmuls contract over the 64 live
  partitions of a pair tile (no zero-padding needed).
- The attention inner loop is paced by the Scalar engine's EXP;
  independent matmul chains (v projection, later pairs' qkv, output
  projection) are injected as fillers between S-groups so the PE never
  idles on exp.  S matmuls and exp are causally trimmed.
- Output stores are deferred to unit boundaries so they never block the
  in-order DMA ring ahead of the normalize broadcasts.
"""

import numpy as np
import ml_dtypes

_BF16 = ml_dtypes.bfloat16

B, T, C = 4, 2048, 768
H, HD = 12, 64
NCORES = 8
NH = 6            # heads per core
CQ = NH * HD      # 384
CPAD = 768        # contraction dim
TQ = 512          # query chunk
KCB = 128         # key block
SGRP = 2          # key blocks per PSUM S-group (2 banks)
VSTR = NH * (HD + 1)  # 390: v_sb stride per key block (65 per head)

_cache = {}


def _build():
    import concourse.bacc as bacc
    import concourse.bass as bass
    from concourse import mybir
    from concourse.tile import TileContext

    f32 = mybir.dt.float32
    bf16 = mybir.dt.bfloat16
    EXP = mybir.ActivationFunctionType.Exp

    nc = bacc.Bacc("TRN2", target_bir_lowering=False, debug=False)
    d_xt = nc.dram_tensor("xt", [CPAD, T], bf16, kind="ExternalInput")
    d_wqk = nc.dram_tensor("wqk", [CPAD, 2 * CQ], bf16, kind="ExternalInput")
    d_wv = nc.dram_tensor("wv", [CPAD, CQ], bf16, kind="ExternalInput")
    d_wp = nc.dram_tensor("wp", [CQ, C], bf16, kind="ExternalInput")
    d_mk = nc.dram_tensor("mk", [KCB, KCB], bf16, kind="ExternalInput")
    d_bqk = nc.dram_tensor("bqk", [128, 6], f32, kind="ExternalInput")
    d_bv = nc.dram_tensor("bv", [1, CQ], f32, kind="ExternalInput")
    d_out = nc.dram_tensor("out", [T, C], f32, kind="ExternalOutput")

    NQC = T // TQ      # 4 query chunks
    NCC = CPAD // 128  # 6 contraction chunks
    NKC = T // KCB     # 16 key blocks

    with TileContext(nc) as tc:
        with tc.tile_pool(name="const", bufs=1) as const:
            # persistent SBUF tiles
            xt_sb = [const.tile([128, T], bf16, name=f"xt{i}", tag=f"xt{i}") for i in range(NCC)]
            wqk_sb = [const.tile([128, 2 * CQ], bf16, name=f"wqk{i}", tag=f"wqk{i}") for i in range(NCC)]
            wv_sb = [const.tile([128, CQ], bf16, name=f"wv{i}", tag=f"wv{i}") for i in range(NCC)]
            wp_sb = [const.tile([128, C], bf16, name=f"wp{i}", tag=f"wp{i}") for i in range(3)]
            mask_sb = const.tile([KCB, KCB], bf16, name="mask", tag="mask")
            # Q^T/K^T per (head pair j, token chunk t4); head 2j in rows
            # 0:64, head 2j+1 in rows 64:128.
            qp_t = [[const.tile([128, TQ], bf16, name=f"q{j}_{t4}", tag=f"q{j}_{t4}")
                     for t4 in range(NQC)] for j in range(3)]
            kp_t = [[const.tile([128, TQ], bf16, name=f"k{j}_{t4}", tag=f"k{j}_{t4}")
                     for t4 in range(NQC)] for j in range(3)]
            v_sb = const.tile([128, NKC * VSTR], bf16, name="v", tag="v")
            yn_sb = [const.tile([128, T], bf16, name=f"yn{i}", tag=f"yn{i}") for i in range(3)]
            bqk_sb = const.tile([128, 6], f32, name="bqk", tag="bqk")
            bvb_sb = const.tile([128, CQ], f32, name="bvb", tag="bvb")

            # input DMAs: one HW ring, FIFO -> emit in first-use order.
            nc.sync.dma_start(out=bqk_sb, in_=d_bqk.ap())
            nc.sync.dma_start(out=mask_sb, in_=d_mk.ap())
            for i in range(4):
                nc.sync.dma_start(out=xt_sb[i], in_=d_xt.ap()[128 * i:128 * (i + 1), :])
                nc.sync.dma_start(out=wqk_sb[i], in_=d_wqk.ap()[128 * i:128 * (i + 1), :])
            for i in range(NCC):
                nc.sync.dma_start(out=wv_sb[i], in_=d_wv.ap()[128 * i:128 * (i + 1), :])
            nc.sync.dma_start(
                out=bvb_sb,
                in_=bass.AP(tensor=d_bv, offset=0, ap=[[0, 128], [1, CQ]]))
            for i in range(4, NCC):
                nc.sync.dma_start(out=xt_sb[i], in_=d_xt.ap()[128 * i:128 * (i + 1), :])
                nc.sync.dma_start(out=wqk_sb[i], in_=d_wqk.ap()[128 * i:128 * (i + 1), :])
            # wp is emitted later (only needed by proj) to keep the ring clear
            # for the mid-kernel normalize broadcasts.

            # ones column per (key block, head) in v_sb
            v_ones = v_sb.rearrange("p (kc h e) -> p kc h e", h=NH, e=HD + 1)[:, :, :, HD:HD + 1]
            nc.gpsimd.memset(v_ones, 1.0)

            def qkv_bias_copy(jc, t4, ps, pcols):
                arr = qp_t if jc < 3 else kp_t
                nc.vector.tensor_scalar_add(
                    arr[jc % 3][t4], ps[:, pcols], bqk_sb[:, jc:jc + 1])

            # boot: qkv for head pair 0 (jc=0 -> Q heads 0,1; jc=3 -> K),
            # contraction-chunk OUTER so the PE eats chunks as they arrive.
            with tc.tile_pool(name="boot", bufs=1, space="PSUM") as bootp:
                btiles = [bootp.tile([128, 2 * TQ], f32, name=f"bt{t4}", tag=f"bt{t4}")
                          for t4 in range(NQC)]
                for cc in range(NCC):
                    for jj, jc in enumerate((0, 3)):
                        for t4 in range(NQC):
                            nc.tensor.matmul(
                                btiles[t4][:, TQ * jj:TQ * (jj + 1)],
                                lhsT=wqk_sb[cc][:, 128 * jc:128 * (jc + 1)],
                                rhs=xt_sb[cc][:, TQ * t4:TQ * (t4 + 1)],
                                start=(cc == 0), stop=(cc == NCC - 1),
                            )
                for t4 in range(NQC):
                    for jj, jc in enumerate((0, 3)):
                        qkv_bias_copy(jc, t4, btiles[t4],
                                      slice(TQ * jj, TQ * (jj + 1)))

            # main pools: work (qkv/v/proj accum) + s + y = 8 PSUM banks
            with tc.tile_pool(name="work", bufs=2, space="PSUM") as work, \
                 tc.tile_pool(name="ps_s", bufs=2, space="PSUM") as ps_s, \
                 tc.tile_pool(name="ps_y", bufs=2, space="PSUM") as ps_y, \
                 tc.tile_pool(name="pp", bufs=4) as pp, \
                 tc.tile_pool(name="smalls", bufs=4) as smalls, \
                 tc.tile_pool(name="outp", bufs=4) as outp:

                # ---- filler chains: independent PE work injected between
                # S-groups so the PE never waits on the Scalar exp. ----
                vdone = [False] * NKC

                def emit_vblk(kc):
                    if vdone[kc]:
                        return False
                    vdone[kc] = True
                    psv = work.tile([128, TQ], f32, name="psv", tag="w")
                    for cc in range(NCC):
                        nc.tensor.matmul(
                            psv[:, 0:CQ],
                            lhsT=xt_sb[cc][:, 128 * kc:128 * (kc + 1)],
                            rhs=wv_sb[cc],
                            start=(cc == 0), stop=(cc == NCC - 1),
                        )
                    dst = v_sb[:, VSTR * kc:VSTR * (kc + 1)].rearrange(
                        "p (h e) -> p h e", e=HD + 1)[:, :, 0:HD]
                    nc.vector.scalar_tensor_tensor(
                        out=dst,
                        in0=psv[:, 0:CQ].rearrange("p (h e) -> p h e", e=HD),
                        scalar=0.0,
                        in1=bvb_sb.rearrange("p (h e) -> p h e", e=HD),
                        op0=mybir.AluOpType.add, op1=mybir.AluOpType.add)
                    return True

                qkvdone = {}

                def emit_qkv_chain(jc, t4):
                    if qkvdone.get((jc, t4)):
                        return False
                    qkvdone[(jc, t4)] = True
                    ps = work.tile([128, TQ], f32, name="qkv", tag="w")
                    for cc in range(NCC):
                        nc.tensor.matmul(
                            ps[:, 0:TQ],
                            lhsT=wqk_sb[cc][:, 128 * jc:128 * (jc + 1)],
                            rhs=xt_sb[cc][:, TQ * t4:TQ * (t4 + 1)],
                            start=(cc == 0), stop=(cc == NCC - 1),
                        )
                    qkv_bias_copy(jc, t4, ps, slice(0, TQ))
                    return True

                projdone = [False] * (T // 128)
                pending_stores = []

                def emit_proj(tcb, pool):
                    if projdone[tcb]:
                        return False
                    projdone[tcb] = True
                    ob = outp.tile([128, C], f32, name="ob", tag="ob")
                    for oc in range(2):
                        po = pool.tile([128, TQ], f32, name="po",
                                       tag="w" if pool is work else "y")
                        for fcc in range(3):
                            nc.tensor.matmul(
                                po[:, 0:CQ],
                                lhsT=yn_sb[fcc][:, 128 * tcb:128 * (tcb + 1)],
                                rhs=wp_sb[fcc][:, CQ * oc:CQ * (oc + 1)],
                                start=(fcc == 0), stop=(fcc == 2),
                            )
                        nc.vector.tensor_copy(ob[:, CQ * oc:CQ * (oc + 1)], po[:, 0:CQ])
                    pending_stores.append((tcb, ob))
                    return True

                def flush_stores():
                    while pending_stores:
                        tcb, ob = pending_stores.pop(0)
                        nc.sync.dma_start(
                            out=d_out.ap()[128 * tcb:128 * tcb + 64, :],
                            in_=ob[0:64, :])
                        nc.sync.dma_start(
                            out=d_out.ap()[128 * tcb + 64:128 * (tcb + 1), :],
                            in_=ob[64:128, :])

                fillq = []

                def fill(n=1):
                    done = 0
                    while done < n and fillq:
                        if fillq.pop(0)():
                            done += 1

                # ---- one attention unit: head h, query chunk qi ----
                def emit_attn_unit(h, qi):
                    flush_stores()  # stores issue while the ring is quiet
                    j, hl = h // 2, slice(0, 64) if h % 2 == 0 else slice(64, 128)
                    q0 = TQ * qi
                    nkc = (q0 + TQ) // KCB  # causal: key blocks 0..nkc-1
                    y = ps_y.tile([HD + 1, TQ], f32, name="y", tag="y")
                    pend = []  # attV lags one S-group behind (sw pipeline)

                    def emit_attv(p, kcs):
                        for idx, kc in enumerate(kcs):
                            r = kc - 4 * qi
                            off = KCB * r if r >= 0 else 0
                            nc.tensor.matmul(
                                y[:, off:TQ],
                                lhsT=v_sb[:, VSTR * kc + (HD + 1) * h:
                                          VSTR * kc + (HD + 1) * (h + 1)],
                                rhs=p[:, TQ * idx + off:TQ * (idx + 1)],
                                start=(kc == 0), stop=(kc == nkc - 1),
                            )

                    for g0 in range(0, nkc, SGRP):
                        kcs = list(range(g0, min(g0 + SGRP, nkc)))
                        s = ps_s.tile([128, SGRP * TQ], f32, name="s", tag="s")
                        offs = []
                        for idx, kc in enumerate(kcs):
                            r = kc - 4 * qi
                            off = KCB * r if r >= 0 else 0
                            offs.append(off)
                            nc.tensor.matmul(
                                s[:, TQ * idx + off:TQ * (idx + 1)],
                                lhsT=kp_t[j][kc // 4][hl, KCB * (kc % 4):KCB * (kc % 4 + 1)],
                                rhs=qp_t[j][qi][hl, off:TQ],
                                start=True, stop=True,
                            )
                        p = pp.tile([128, SGRP * TQ], bf16, name="p", tag="p")
                        if offs[-1] == 0:  # no trim in this group
                            w = len(kcs) * TQ
                            nc.scalar.activation(p[:, 0:w], s[:, 0:w], EXP)
                        else:  # diagonal group: exp only the live region
                            for idx in range(len(kcs)):
                                lo, hi = TQ * idx + offs[idx], TQ * (idx + 1)
                                nc.scalar.activation(p[:, lo:hi], s[:, lo:hi], EXP)
                        for idx, kc in enumerate(kcs):
                            r = kc - 4 * qi
                            if r >= 0:  # diagonal block: apply triangle mask
                                lo = TQ * idx + KCB * r
                                nc.gpsimd.tensor_mul(
                                    p[:, lo:lo + KCB], p[:, lo:lo + KCB], mask_sb)
                        pend.append((p, kcs))
                        if len(pend) > 1:
                            emit_attv(*pend.pop(0))
                        fill(1)
                    emit_attv(*pend.pop(0))
                    # normalize by softmax denominator (row HD of y)
                    rc = smalls.tile([HD + 1, TQ], f32, name="rc", tag="rc")
                    nc.vector.tensor_copy(rc[HD:HD + 1, :], y[HD:HD + 1, :])
                    rb = smalls.tile([HD, TQ], f32, name="rb", tag="rb")
                    lsrc = rc[HD:HD + 1, :]
                    for qd in range(4):  # 4 sem lanes: descriptor-latency /4
                        nc.sync.dma_start(
                            out=rb[16 * qd:16 * (qd + 1), :],
                            in_=bass.AP(tensor=lsrc.tensor, offset=lsrc.offset,
                                        ap=[lsrc.ap[0], [0, 16], lsrc.ap[1]]))
                    nc.vector.reciprocal_approx_fast(out=rb, in_=rb)
                    fc, half = h // 2, h % 2
                    if half == 0:
                        nc.vector.tensor_mul(
                            yn_sb[fc][0:HD, q0:q0 + TQ], y[0:HD, :], rb)
                    else:
                        tt = smalls.tile([HD, TQ], bf16, name="tt", tag="tt")
                        nc.vector.tensor_mul(tt, y[0:HD, :], rb)
                        nc.sync.dma_start(
                            out=yn_sb[fc][HD:2 * HD, q0:q0 + TQ], in_=tt)

                # v blocks 0..3 must exist before the first attention unit
                for kc in range(4):
                    emit_vblk(kc)
                # filler queue: rest of v, then qkv for pairs 1 and 2
                for kc in range(4, NKC):
                    fillq.append(lambda kc=kc: emit_vblk(kc))
                for jc in (1, 4, 2, 5):
                    for t4 in range(NQC):
                        fillq.append(lambda jc=jc, t4=t4: emit_qkv_chain(jc, t4))

                # pairs 0 and 1: head-outer
                for pair in range(2):
                    for h in (2 * pair, 2 * pair + 1):
                        for qi in range(NQC):
                            for kc in range(4 * qi + 4):
                                emit_vblk(kc)
                            emit_attn_unit(h, qi)
                    # next pair's q/k must be ready before its first unit
                    for jc in (pair + 1, pair + 4):
                        for t4 in range(NQC):
                            emit_qkv_chain(jc, t4)
                    if pair == 0:  # load wp now: ring is quiet, proj is later
                        for i in range(3):
                            nc.sync.dma_start(
                                out=wp_sb[i], in_=d_wp.ap()[128 * i:128 * (i + 1), :])
                # pair 2: qi-outer so proj column blocks release progressively
                for qi in range(NQC):
                    for h in (4, 5):
                        emit_attn_unit(h, qi)
                    for tcb in range(4 * qi, 4 * qi + 4):
                        fillq.append(lambda tcb=tcb: emit_proj(tcb, work))
                # drain remaining proj (deeper pipeline via the now-idle ps_y)
                for tcb in range(T // 128):
                    emit_proj(tcb, ps_y)
                    flush_stores()

    nc.compile()
    return nc


def _prep_core(x, w_attn, b_attn, w_proj, c):
    b, g = c // 2, c % 2
    h0 = NH * g
    q = slice(64 * h0, 64 * h0 + CQ)
    k = slice(C + 64 * h0, C + 64 * h0 + CQ)
    v = slice(2 * C + 64 * h0, 2 * C + 64 * h0 + CQ)

    xt = np.ascontiguousarray(x[b].T).astype(_BF16)

    wqk = np.empty((CPAD, 2 * CQ), dtype=_BF16)
    wqk[:, 0:CQ] = (w_attn[:, q] * 0.125).astype(_BF16)
    wqk[:, CQ:] = w_attn[:, k].astype(_BF16)
    bqk = np.concatenate([b_attn[q] * 0.125, b_attn[k]]).astype(np.float32)
    bqk = np.ascontiguousarray(bqk.reshape(6, 128).T)

    wv = np.ascontiguousarray(w_attn[:, v]).astype(_BF16)
    bv = np.ascontiguousarray(b_attn[v].reshape(1, CQ)).astype(np.float32)

    wp = np.ascontiguousarray(w_proj[q, :]).astype(_BF16)

    ii = np.arange(KCB)
    mk = (ii[:, None] <= ii[None, :]).astype(_BF16)
    return {"xt": xt, "wqk": wqk, "wv": wv, "wp": wp, "mk": mk, "bqk": bqk,
            "bv": bv}


def kernel(x, w_attn, b_attn, w_proj, b_proj):
    from concourse.bass_utils import run_bass_kernel_spmd

    x = np.asarray(x, dtype=np.float32)
    w_attn = np.asarray(w_attn, dtype=np.float32)
    b_attn = np.asarray(b_attn, dtype=np.float32)
    w_proj = np.asarray(w_proj, dtype=np.float32)
    b_proj = np.asarray(b_proj, dtype=np.float32)

    if "nc" not in _cache:
        _cache["nc"] = _build()
    nc = _cache["nc"]

    in_maps = [_prep_core(x, w_attn, b_attn, w_proj, c) for c in range(NCORES)]
    res = run_bass_kernel_spmd(nc, in_maps, core_ids=list(range(NCORES)))

    out = np.empty((B, T, C), dtype=np.float32)
    for b in range(B):
        out[b] = res.results[2 * b]["out"] + res.results[2 * b + 1]["out"] + b_proj
    return out


# revision 8
# speedup vs baseline: 1.0389x; 1.0389x over previous
"""Causal self-attention (B=4, T=2048, C=768, H=12) on 8 trn2 NeuronCores.

Sharding: core c -> batch c//2, head-group c%2 (6 heads each).
Each core computes qkv projection, flash-style causal attention and its
partial c_proj contribution for its 6 heads; the host sums the two
head-group partials per batch and adds b_proj.

Schedule notes:
- All DMAs share one HW ring (FIFO): loads are emitted in first-use
  order and the boot qkv projection runs contraction-chunk-outer so the
  PE consumes input chunks as they arrive.
- Q^T/K^T live in per-(head, 512-token-chunk) tiles so Tile's per-tile
  dependency tracking gates each attention unit only on the chunk it
  reads.  Each tile keeps the head in its 64-row half and ZEROS in the
  other half, written together by one fused tensor_scalar
  (psum*mask + masked_bias): S matmuls then contract over the full 128
  partitions, which keeps the PE HAM activity monitor at full clock
  (64-row contraction measurably re-throttles the PE to 1.2 GHz).
- The attention inner loop is paced by the Scalar engine's EXP;
  independent matmul chains (v blocks, later pairs' qkv, output proj)
  are injected as fillers between S-groups so the PE never idles.
- Output stores are deferred to unit boundaries (and held entirely
  during the last query chunk) so they never sit in the in-order DMA
  ring ahead of the softmax-normalize broadcasts.
"""

import numpy as np
import ml_dtypes

_BF16 = ml_dtypes.bfloat16

B, T, C = 4, 2048, 768
H, HD = 12, 64
NCORES = 8
NH = 6            # heads per core
CQ = NH * HD      # 384
CPAD = 768        # contraction dim
TQ = 512          # query chunk
KCB = 128         # key block
SGRP = 2          # key blocks per PSUM S-group (2 banks)
VSTR = NH * (HD + 1)  # 390: v_sb stride per key block (65 per head)

_cache = {}


def _build():
    import concourse.bacc as bacc
    import concourse.bass as bass
    from concourse import mybir
    from concourse.tile import TileContext

    f32 = mybir.dt.float32
    bf16 = mybir.dt.bfloat16
    EXP = mybir.ActivationFunctionType.Exp

    nc = bacc.Bacc("TRN2", target_bir_lowering=False, debug=False)
    d_xt = nc.dram_tensor("xt", [CPAD, T], bf16, kind="ExternalInput")
    d_wqk = nc.dram_tensor("wqk", [CPAD, 2 * CQ], bf16, kind="ExternalInput")
    d_wv = nc.dram_tensor("wv", [CPAD, CQ], bf16, kind="ExternalInput")
    d_wp = nc.dram_tensor("wp", [CQ, C], bf16, kind="ExternalInput")
    d_mk = nc.dram_tensor("mk", [KCB, KCB], bf16, kind="ExternalInput")
    d_bqk = nc.dram_tensor("bqk", [128, 14], f32, kind="ExternalInput")
    d_bv = nc.dram_tensor("bv", [1, CQ], f32, kind="ExternalInput")
    d_out = nc.dram_tensor("out", [T, C], f32, kind="ExternalOutput")

    NQC = T // TQ      # 4 query chunks
    NCC = CPAD // 128  # 6 contraction chunks
    NKC = T // KCB     # 16 key blocks

    with TileContext(nc) as tc:
        with tc.tile_pool(name="const", bufs=1) as const:
            # persistent SBUF tiles
            xt_sb = [const.tile([128, T], bf16, name=f"xt{i}", tag=f"xt{i}") for i in range(NCC)]
            wqk_sb = [const.tile([128, 2 * CQ], bf16, name=f"wqk{i}", tag=f"wqk{i}") for i in range(NCC)]
            wv_sb = [const.tile([128, CQ], bf16, name=f"wv{i}", tag=f"wv{i}") for i in range(NCC)]
            wp_sb = [const.tile([128, C], bf16, name=f"wp{i}", tag=f"wp{i}") for i in range(3)]
            mask_sb = const.tile([KCB, KCB], bf16, name="mask", tag="mask")
            # Q^T/K^T per (head h, token chunk t4): head in rows 0:64 (even h)
            # or 64:128 (odd h), zeros in the other half.
            q_t = [[const.tile([128, TQ], bf16, name=f"q{h}_{t4}", tag=f"q{h}_{t4}")
                    for t4 in range(NQC)] for h in range(NH)]
            k_t = [[const.tile([128, TQ], bf16, name=f"k{h}_{t4}", tag=f"k{h}_{t4}")
                    for t4 in range(NQC)] for h in range(NH)]
            v_sb = const.tile([128, NKC * VSTR], bf16, name="v", tag="v")
            yn_sb = [const.tile([128, T], bf16, name=f"yn{i}", tag=f"yn{i}") for i in range(3)]
            bqk_sb = const.tile([128, 14], f32, name="bqk", tag="bqk")
            bvb_sb = const.tile([128, CQ], f32, name="bvb", tag="bvb")

            # input DMAs: one HW ring, FIFO -> emit in first-use order.
            nc.sync.dma_start(out=bqk_sb, in_=d_bqk.ap())
            nc.sync.dma_start(out=mask_sb, in_=d_mk.ap())
            for i in range(4):
                nc.sync.dma_start(out=xt_sb[i], in_=d_xt.ap()[128 * i:128 * (i + 1), :])
                nc.sync.dma_start(out=wqk_sb[i], in_=d_wqk.ap()[128 * i:128 * (i + 1), :])
            for i in range(NCC):
                nc.sync.dma_start(out=wv_sb[i], in_=d_wv.ap()[128 * i:128 * (i + 1), :])
            nc.sync.dma_start(
                out=bvb_sb,
                in_=bass.AP(tensor=d_bv, offset=0, ap=[[0, 128], [1, CQ]]))
            for i in range(4, NCC):
                nc.sync.dma_start(out=xt_sb[i], in_=d_xt.ap()[128 * i:128 * (i + 1), :])
                nc.sync.dma_start(out=wqk_sb[i], in_=d_wqk.ap()[128 * i:128 * (i + 1), :])
            # wp is emitted later (only needed by proj) to keep the ring clear
            # for the mid-kernel normalize broadcasts.

            # ones column per (key block, head) in v_sb
            v_ones = v_sb.rearrange("p (kc h e) -> p kc h e", h=NH, e=HD + 1)[:, :, :, HD:HD + 1]
            nc.gpsimd.memset(v_ones, 1.0)

            def qkv_bias_copy(jc, t4, ps, pcols):
                # one fused op per head tile: out = ps*halfmask + masked_bias
                # (writes the head's half AND zeros the dead half)
                arr = q_t if jc < 3 else k_t
                j = jc % 3
                for half in range(2):
                    nc.vector.tensor_scalar(
                        out=arr[2 * j + half][t4],
                        in0=ps[:, pcols],
                        scalar1=bqk_sb[:, 12 + half:13 + half],
                        scalar2=bqk_sb[:, 2 * jc + half:2 * jc + half + 1],
                        op0=mybir.AluOpType.mult,
                        op1=mybir.AluOpType.add)

            # work pool is opened before boot so v-block chains can
            # interleave with the boot bias-copies.
            with tc.tile_pool(name="work", bufs=2, space="PSUM") as work:

                vdone = [False] * NKC

                def emit_vblk(kc):
                    if vdone[kc]:
                        return False
                    vdone[kc] = True
                    psv = work.tile([128, TQ], f32, name="psv", tag="w")
                    for cc in range(NCC):
                        nc.tensor.matmul(
                            psv[:, 0:CQ],
                            lhsT=xt_sb[cc][:, 128 * kc:128 * (kc + 1)],
                            rhs=wv_sb[cc],
                            start=(cc == 0), stop=(cc == NCC - 1),
                        )
                    dst = v_sb[:, VSTR * kc:VSTR * (kc + 1)].rearrange(
                        "p (h e) -> p h e", e=HD + 1)[:, :, 0:HD]
                    nc.vector.scalar_tensor_tensor(
                        out=dst,
                        in0=psv[:, 0:CQ].rearrange("p (h e) -> p h e", e=HD),
                        scalar=0.0,
                        in1=bvb_sb.rearrange("p (h e) -> p h e", e=HD),
                        op0=mybir.AluOpType.add, op1=mybir.AluOpType.add)
                    return True

                qkvdone = {}

                def emit_qkv_chain(jc, t4):
                    if qkvdone.get((jc, t4)):
                        return False
                    qkvdone[(jc, t4)] = True
                    ps = work.tile([128, TQ], f32, name="qkv", tag="w")
                    for cc in range(NCC):
                        nc.tensor.matmul(
                            ps[:, 0:TQ],
                            lhsT=wqk_sb[cc][:, 128 * jc:128 * (jc + 1)],
                            rhs=xt_sb[cc][:, TQ * t4:TQ * (t4 + 1)],
                            start=(cc == 0), stop=(cc == NCC - 1),
                        )
                    qkv_bias_copy(jc, t4, ps, slice(0, TQ))
                    return True

                # boot: qkv for head pair 0, t4 chunks 0..2 (6 PSUM banks,
                # coexisting with work's 2), contraction-chunk OUTER so the
                # PE eats input chunks as they arrive.  t4=3 goes through
                # the regular filler path.
                NBT = 3
                with tc.tile_pool(name="boot", bufs=1, space="PSUM") as bootp:
                    btiles = [bootp.tile([128, 2 * TQ], f32, name=f"bt{t4}", tag=f"bt{t4}")
                              for t4 in range(NBT)]
                    for cc in range(NCC):
                        for t4 in range(NBT):
                            for jj, jc in enumerate((0, 3)):
                                nc.tensor.matmul(
                                    btiles[t4][:, TQ * jj:TQ * (jj + 1)],
                                    lhsT=wqk_sb[cc][:, 128 * jc:128 * (jc + 1)],
                                    rhs=xt_sb[cc][:, TQ * t4:TQ * (t4 + 1)],
                                    start=(cc == 0), stop=(cc == NCC - 1),
                                )
                    for t4 in range(NBT):
                        qkvdone[(0, t4)] = qkvdone[(3, t4)] = True
                        for jj, jc in enumerate((0, 3)):
                            qkv_bias_copy(jc, t4, btiles[t4],
                                          slice(TQ * jj, TQ * (jj + 1)))
                        if t4 == 0:      # v0/v1 interleave into the DVE queue
                            emit_vblk(0)  # right behind the h0q0 gate copies
                            emit_vblk(1)
                        elif t4 == 1:
                            emit_vblk(2)
                            emit_vblk(3)

                # attention pools: 2+4+2 = 8 PSUM banks with work
                with tc.tile_pool(name="ps_s", bufs=2, space="PSUM") as ps_s, \
                     tc.tile_pool(name="ps_y", bufs=2, space="PSUM") as ps_y, \
                     tc.tile_pool(name="pp", bufs=4) as pp, \
                     tc.tile_pool(name="smalls", bufs=4) as smalls, \
                     tc.tile_pool(name="outp", bufs=4) as outp:

                    projdone = [False] * (T // 128)
                    pending_stores = []

                    def emit_proj(tcb):
                        if projdone[tcb]:
                            return False
                        projdone[tcb] = True
                        ob = outp.tile([128, C], f32, name="ob", tag="ob")
                        for oc in range(2):
                            po = work.tile([128, TQ], f32, name="po", tag="w")
                            for fcc in range(3):
                                nc.tensor.matmul(
                                    po[:, 0:CQ],
                                    lhsT=yn_sb[fcc][:, 128 * tcb:128 * (tcb + 1)],
                                    rhs=wp_sb[fcc][:, CQ * oc:CQ * (oc + 1)],
                                    start=(fcc == 0), stop=(fcc == 2),
                                )
                            nc.vector.tensor_copy(ob[:, CQ * oc:CQ * (oc + 1)], po[:, 0:CQ])
                        pending_stores.append((tcb, ob))
                        return True

                    def flush_stores():
                        while pending_stores:
                            tcb, ob = pending_stores.pop(0)
                            nc.sync.dma_start(
                                out=d_out.ap()[128 * tcb:128 * tcb + 64, :],
                                in_=ob[0:64, :])
                            nc.sync.dma_start(
                                out=d_out.ap()[128 * tcb + 64:128 * (tcb + 1), :],
                                in_=ob[64:128, :])

                    fillq = []

                    def fill(n=1):
                        done = 0
                        while done < n and fillq:
                            if fillq.pop(0)():
                                done += 1

                    # ---- one attention unit: head h, query chunk qi ----
                    def emit_attn_unit(h, qi, hold_stores=False):
                        if not hold_stores:
                            flush_stores()
                        j = h // 2
                        # q/k for this unit (no-ops when already emitted)
                        emit_qkv_chain(j, qi)
                        emit_qkv_chain(3 + j, qi)
                        for kc in range(4 * qi + 4):
                            emit_vblk(kc)
                        q0 = TQ * qi
                        nkc = (q0 + TQ) // KCB
                        y = ps_y.tile([HD + 1, TQ], f32, name="y", tag="y")
                        pend = []  # attV lags one S-group behind

                        def emit_attv(p, kcs):
                            for idx, kc in enumerate(kcs):
                                r = kc - 4 * qi
                                off = KCB * r if r >= 0 else 0
                                nc.tensor.matmul(
                                    y[:, off:TQ],
                                    lhsT=v_sb[:, VSTR * kc + (HD + 1) * h:
                                              VSTR * kc + (HD + 1) * (h + 1)],
                                    rhs=p[:, TQ * idx + off:TQ * (idx + 1)],
                                    start=(kc == 0), stop=(kc == nkc - 1),
                                )

                        for g0 in range(0, nkc, SGRP):
                            kcs = list(range(g0, min(g0 + SGRP, nkc)))
                            s = ps_s.tile([128, SGRP * TQ], f32, name="s", tag="s")
                            offs = []
                            for idx, kc in enumerate(kcs):
                                r = kc - 4 * qi
                                off = KCB * r if r >= 0 else 0
                                offs.append(off)
                                nc.tensor.matmul(
                                    s[:, TQ * idx + off:TQ * (idx + 1)],
                                    lhsT=k_t[h][kc // 4][:, KCB * (kc % 4):KCB * (kc % 4 + 1)],
                                    rhs=q_t[h][qi][:, off:TQ],
                                    start=True, stop=True,
                                )
                            p = pp.tile([128, SGRP * TQ], bf16, name="p", tag="p")
                            if offs[-1] == 0:  # no trim in this group
                                w = len(kcs) * TQ
                                nc.scalar.activation(p[:, 0:w], s[:, 0:w], EXP)
                            else:  # diagonal group: exp only the live region
                                for idx in range(len(kcs)):
                                    lo, hi = TQ * idx + offs[idx], TQ * (idx + 1)
                                    nc.scalar.activation(p[:, lo:hi], s[:, lo:hi], EXP)
                            for idx, kc in enumerate(kcs):
                                r = kc - 4 * qi
                                if r >= 0:  # diagonal block: triangle mask
                                    lo = TQ * idx + KCB * r
                                    nc.gpsimd.tensor_mul(
                                        p[:, lo:lo + KCB], p[:, lo:lo + KCB], mask_sb)
                            pend.append((p, kcs))
                            if len(pend) > 1:
                                emit_attv(*pend.pop(0))
                            fill(1)
                        emit_attv(*pend.pop(0))
                        # normalize by softmax denominator (row HD of y)
                        rc = smalls.tile([HD + 1, TQ], f32, name="rc", tag="rc")
                        nc.vector.tensor_copy(rc[HD:HD + 1, :], y[HD:HD + 1, :])
                        rb = smalls.tile([HD, TQ], f32, name="rb", tag="rb")
                        lsrc = rc[HD:HD + 1, :]
                        for qd in range(4):
                            nc.sync.dma_start(
                                out=rb[16 * qd:16 * (qd + 1), :],
                                in_=bass.AP(tensor=lsrc.tensor, offset=lsrc.offset,
                                            ap=[lsrc.ap[0], [0, 16], lsrc.ap[1]]))
                        nc.vector.reciprocal_approx_fast(out=rb, in_=rb)
                        fc, half = h // 2, h % 2
                        if half == 0:
                            nc.vector.tensor_mul(
                                yn_sb[fc][0:HD, q0:q0 + TQ], y[0:HD, :], rb)
                        else:
                            tt = smalls.tile([HD, TQ], bf16, name="tt", tag="tt")
                            nc.vector.tensor_mul(tt, y[0:HD, :], rb)
                            nc.sync.dma_start(
                                out=yn_sb[fc][HD:2 * HD, q0:q0 + TQ], in_=tt)

                    # filler queue: pair-0 t4=3 qkv, rest of v, later pairs
                    fillq.append(lambda: emit_qkv_chain(0, 3))
                    fillq.append(lambda: emit_qkv_chain(3, 3))
                    for kc in range(4, NKC):
                        fillq.append(lambda kc=kc: emit_vblk(kc))
                    for jc in (1, 4, 2, 5):
                        for t4 in range(NQC):
                            fillq.append(lambda jc=jc, t4=t4: emit_qkv_chain(jc, t4))

                    # pairs 0 and 1: head-outer
                    for pair in range(2):
                        for h in (2 * pair, 2 * pair + 1):
                            for qi in range(NQC):
                                emit_attn_unit(h, qi)
                        # next pair's q/k ready before its first unit
                        for jc in (pair + 1, pair + 4):
                            for t4 in range(NQC):
                                emit_qkv_chain(jc, t4)
                        if pair == 0:  # wp load: ring quiet, proj far away
                            for i in range(3):
                                nc.sync.dma_start(
                                    out=wp_sb[i], in_=d_wp.ap()[128 * i:128 * (i + 1), :])
                    # pair 2: qi-outer so proj column blocks release early;
                    # stores held during the last chunk so the final
                    # normalize broadcasts aren't stuck behind them.
                    for qi in range(NQC):
                        for h in (4, 5):
                            emit_attn_unit(h, qi, hold_stores=(qi == NQC - 1))
                        for tcb in range(4 * qi, 4 * qi + 4):
                            fillq.append(lambda tcb=tcb: emit_proj(tcb))
                    # drain remaining proj + stores
                    for tcb in range(T // 128):
                        emit_proj(tcb)
                        flush_stores()

    nc.compile()
    return nc


def _prep_core(x, w_attn, b_attn, w_proj, c):
    b, g = c // 2, c % 2
    h0 = NH * g
    q = slice(64 * h0, 64 * h0 + CQ)
    k = slice(C + 64 * h0, C + 64 * h0 + CQ)
    v = slice(2 * C + 64 * h0, 2 * C + 64 * h0 + CQ)

    xt = np.ascontiguousarray(x[b].T).astype(_BF16)

    wqk = np.empty((CPAD, 2 * CQ), dtype=_BF16)
    wqk[:, 0:CQ] = (w_attn[:, q] * 0.125).astype(_BF16)
    wqk[:, CQ:] = w_attn[:, k].astype(_BF16)
    bqk6 = np.concatenate([b_attn[q] * 0.125, b_attn[k]]).astype(np.float32)
    bqk6 = np.ascontiguousarray(bqk6.reshape(6, 128).T)  # [128, 6]
    # masked biases + half-masks: col 2jc (+1) = bias for even (odd) head
    # half; cols 12/13 = the 0/1 half-masks themselves.
    m0 = np.zeros((128, 1), np.float32); m0[0:64] = 1.0
    m1 = 1.0 - m0
    bqk = np.empty((128, 14), np.float32)
    for jc in range(6):
        bqk[:, 2 * jc:2 * jc + 1] = bqk6[:, jc:jc + 1] * m0
        bqk[:, 2 * jc + 1:2 * jc + 2] = bqk6[:, jc:jc + 1] * m1
    bqk[:, 12:13] = m0
    bqk[:, 13:14] = m1

    wv = np.ascontiguousarray(w_attn[:, v]).astype(_BF16)
    bv = np.ascontiguousarray(b_attn[v].reshape(1, CQ)).astype(np.float32)

    wp = np.ascontiguousarray(w_proj[q, :]).astype(_BF16)

    ii = np.arange(KCB)
    mk = (ii[:, None] <= ii[None, :]).astype(_BF16)
    return {"xt": xt, "wqk": wqk, "wv": wv, "wp": wp, "mk": mk, "bqk": bqk,
            "bv": bv}


def kernel(x, w_attn, b_attn, w_proj, b_proj):
    from concourse.bass_utils import run_bass_kernel_spmd

    x = np.asarray(x, dtype=np.float32)
    w_attn = np.asarray(w_attn, dtype=np.float32)
    b_attn = np.asarray(b_attn, dtype=np.float32)
    w_proj = np.asarray(w_proj, dtype=np.float32)
    b_proj = np.asarray(b_proj, dtype=np.float32)

    if "nc" not in _cache:
        _cache["nc"] = _build()
    nc = _cache["nc"]

    in_maps = [_prep_core(x, w_attn, b_attn, w_proj, c) for c in range(NCORES)]
    res = run_bass_kernel_spmd(nc, in_maps, core_ids=list(range(NCORES)))

    out = np.empty((B, T, C), dtype=np.float32)
    for b in range(B):
        out[b] = res.results[2 * b]["out"] + res.results[2 * b + 1]["out"] + b_proj
    return out


# revision 13
# speedup vs baseline: 1.0685x; 1.0285x over previous
"""Causal self-attention (B=4, T=2048, C=768, H=12) on 8 trn2 NeuronCores.

Sharding: core c -> batch c//2, head-group c%2 (6 heads each).
Each core computes qkv projection, flash-style causal attention and its
partial c_proj contribution for its 6 heads; the host sums the two
head-group partials per batch and adds b_proj.

Schedule notes:
- All DMAs share one HW ring (FIFO): loads are emitted in first-use
  order and the boot qkv projection runs contraction-chunk-outer so the
  PE consumes input chunks as they arrive.
- Q^T/K^T live in per-(head, 512-token-chunk) tiles so Tile's per-tile
  dependency tracking gates each attention unit only on the chunk it
  reads.  Each tile keeps the head in its 64-row half and ZEROS in the
  other half, written together by one fused tensor_scalar
  (psum*mask + masked_bias): S matmuls then contract over the full 128
  partitions, which keeps the PE HAM activity monitor at full clock
  (64-row contraction measurably re-throttles the PE to 1.2 GHz).
- The attention inner loop is paced by the Scalar engine's EXP;
  independent matmul chains (v blocks, later pairs' qkv, output proj)
  are injected as fillers between S-groups so the PE never idles.
- Output stores are deferred to unit boundaries (and held entirely
  during the last query chunk) so they never sit in the in-order DMA
  ring ahead of the softmax-normalize broadcasts.
"""

import numpy as np
import ml_dtypes

_BF16 = ml_dtypes.bfloat16

B, T, C = 4, 2048, 768
H, HD = 12, 64
NCORES = 8
NH = 6            # heads per core
CQ = NH * HD      # 384
CPAD = 768        # contraction dim
TQ = 512          # query chunk
KCB = 128         # key block
SGRP = 2          # key blocks per PSUM S-group (2 banks)
VSTR = NH * (HD + 1)  # 390: v_sb stride per key block (65 per head)

_cache = {}


def _build():
    import concourse.bacc as bacc
    import concourse.bass as bass
    from concourse import mybir
    from concourse.tile import TileContext

    f32 = mybir.dt.float32
    bf16 = mybir.dt.bfloat16
    EXP = mybir.ActivationFunctionType.Exp

    nc = bacc.Bacc("TRN2", target_bir_lowering=False, debug=False)
    d_xt = nc.dram_tensor("xt", [CPAD, T], bf16, kind="ExternalInput")
    d_wqk = nc.dram_tensor("wqk", [CPAD, 2 * CQ], bf16, kind="ExternalInput")
    d_wv = nc.dram_tensor("wv", [CPAD, CQ], bf16, kind="ExternalInput")
    d_wp = nc.dram_tensor("wp", [CQ, C], bf16, kind="ExternalInput")
    d_mk = nc.dram_tensor("mk", [KCB, KCB], bf16, kind="ExternalInput")
    d_bqk = nc.dram_tensor("bqk", [128, 14], f32, kind="ExternalInput")
    d_bv = nc.dram_tensor("bv", [1, CQ], f32, kind="ExternalInput")
    d_out = nc.dram_tensor("out", [T, C], f32, kind="ExternalOutput")

    NQC = T // TQ      # 4 query chunks
    NCC = CPAD // 128  # 6 contraction chunks
    NKC = T // KCB     # 16 key blocks

    with TileContext(nc) as tc:
        with tc.tile_pool(name="const", bufs=1) as const:
            # persistent SBUF tiles
            xt_sb = [const.tile([128, T], bf16, name=f"xt{i}", tag=f"xt{i}") for i in range(NCC)]
            wqk_sb = [const.tile([128, 2 * CQ], bf16, name=f"wqk{i}", tag=f"wqk{i}") for i in range(NCC)]
            wv_sb = [const.tile([128, CQ], bf16, name=f"wv{i}", tag=f"wv{i}") for i in range(NCC)]
            wp_sb = [const.tile([128, C], bf16, name=f"wp{i}", tag=f"wp{i}") for i in range(3)]
            mask_sb = const.tile([KCB, KCB], bf16, name="mask", tag="mask")
            # Q^T/K^T per (head h, token chunk t4): head in rows 0:64 (even h)
            # or 64:128 (odd h), zeros in the other half.
            q_t = [[const.tile([128, TQ], bf16, name=f"q{h}_{t4}", tag=f"q{h}_{t4}")
                    for t4 in range(NQC)] for h in range(NH)]
            k_t = [[const.tile([128, TQ], bf16, name=f"k{h}_{t4}", tag=f"k{h}_{t4}")
                    for t4 in range(NQC)] for h in range(NH)]
            v_sb = const.tile([128, NKC * VSTR], bf16, name="v", tag="v")
            yn_sb = [const.tile([128, T], bf16, name=f"yn{i}", tag=f"yn{i}") for i in range(3)]
            bqk_sb = const.tile([128, 14], f32, name="bqk", tag="bqk")
            bvb_sb = const.tile([128, CQ], f32, name="bvb", tag="bvb")

            # input DMAs: one HW ring, FIFO -> emit in first-use order.
            nc.sync.dma_start(out=bqk_sb, in_=d_bqk.ap())
            nc.sync.dma_start(out=mask_sb, in_=d_mk.ap())
            for i in range(4):
                nc.sync.dma_start(out=xt_sb[i], in_=d_xt.ap()[128 * i:128 * (i + 1), :])
                nc.sync.dma_start(out=wqk_sb[i], in_=d_wqk.ap()[128 * i:128 * (i + 1), :])
            for i in range(NCC):
                nc.sync.dma_start(out=wv_sb[i], in_=d_wv.ap()[128 * i:128 * (i + 1), :])
            nc.sync.dma_start(
                out=bvb_sb,
                in_=bass.AP(tensor=d_bv, offset=0, ap=[[0, 128], [1, CQ]]))
            for i in range(4, NCC):
                nc.sync.dma_start(out=xt_sb[i], in_=d_xt.ap()[128 * i:128 * (i + 1), :])
                nc.sync.dma_start(out=wqk_sb[i], in_=d_wqk.ap()[128 * i:128 * (i + 1), :])
            # wp is emitted later (only needed by proj) to keep the ring clear
            # for the mid-kernel normalize broadcasts.

            # ones column per (key block, head) in v_sb
            v_ones = v_sb.rearrange("p (kc h e) -> p kc h e", h=NH, e=HD + 1)[:, :, :, HD:HD + 1]
            nc.gpsimd.memset(v_ones, 1.0)

            def qkv_bias_copy(jc, t4, ps, pcols):
                # one fused op per head tile: out = ps*halfmask + masked_bias
                # (writes the head's half AND zeros the dead half)
                arr = q_t if jc < 3 else k_t
                j = jc % 3
                for half in range(2):
                    nc.vector.tensor_scalar(
                        out=arr[2 * j + half][t4],
                        in0=ps[:, pcols],
                        scalar1=bqk_sb[:, 12 + half:13 + half],
                        scalar2=bqk_sb[:, 2 * jc + half:2 * jc + half + 1],
                        op0=mybir.AluOpType.mult,
                        op1=mybir.AluOpType.add)

            # work pool is opened before boot so v-block chains can
            # interleave with the boot bias-copies.
            with tc.tile_pool(name="work", bufs=2, space="PSUM") as work:

                vdone = [False] * NKC

                def emit_vblk(kc):
                    if vdone[kc]:
                        return False
                    vdone[kc] = True
                    psv = work.tile([128, TQ], f32, name="psv", tag="w")
                    for cc in range(NCC):
                        nc.tensor.matmul(
                            psv[:, 0:CQ],
                            lhsT=xt_sb[cc][:, 128 * kc:128 * (kc + 1)],
                            rhs=wv_sb[cc],
                            start=(cc == 0), stop=(cc == NCC - 1),
                        )
                    dst = v_sb[:, VSTR * kc:VSTR * (kc + 1)].rearrange(
                        "p (h e) -> p h e", e=HD + 1)[:, :, 0:HD]
                    nc.vector.scalar_tensor_tensor(
                        out=dst,
                        in0=psv[:, 0:CQ].rearrange("p (h e) -> p h e", e=HD),
                        scalar=0.0,
                        in1=bvb_sb.rearrange("p (h e) -> p h e", e=HD),
                        op0=mybir.AluOpType.add, op1=mybir.AluOpType.add)
                    return True

                qkvdone = {}

                def emit_qkv_chain(jc, t4):
                    if qkvdone.get((jc, t4)):
                        return False
                    qkvdone[(jc, t4)] = True
                    ps = work.tile([128, TQ], f32, name="qkv", tag="w")
                    for cc in range(NCC):
                        nc.tensor.matmul(
                            ps[:, 0:TQ],
                            lhsT=wqk_sb[cc][:, 128 * jc:128 * (jc + 1)],
                            rhs=xt_sb[cc][:, TQ * t4:TQ * (t4 + 1)],
                            start=(cc == 0), stop=(cc == NCC - 1),
                        )
                    qkv_bias_copy(jc, t4, ps, slice(0, TQ))
                    return True

                # boot: qkv for head pair 0, t4 chunks 0..2 (6 PSUM banks,
                # coexisting with work's 2), contraction-chunk OUTER so the
                # PE eats input chunks as they arrive.  t4=3 goes through
                # the regular filler path.
                NBT = 3
                with tc.tile_pool(name="boot", bufs=1, space="PSUM") as bootp:
                    btiles = [bootp.tile([128, 2 * TQ], f32, name=f"bt{t4}", tag=f"bt{t4}")
                              for t4 in range(NBT)]
                    for cc in range(NCC):
                        for t4 in range(NBT):
                            for jj, jc in enumerate((0, 3)):
                                nc.tensor.matmul(
                                    btiles[t4][:, TQ * jj:TQ * (jj + 1)],
                                    lhsT=wqk_sb[cc][:, 128 * jc:128 * (jc + 1)],
                                    rhs=xt_sb[cc][:, TQ * t4:TQ * (t4 + 1)],
                                    start=(cc == 0), stop=(cc == NCC - 1),
                                )
                    for t4 in range(NBT):
                        qkvdone[(0, t4)] = qkvdone[(3, t4)] = True
                        for jj, jc in enumerate((0, 3)):
                            qkv_bias_copy(jc, t4, btiles[t4],
                                          slice(TQ * jj, TQ * (jj + 1)))
                        if t4 == 0:      # v0/v1 interleave into the DVE queue
                            emit_vblk(0)  # right behind the h0q0 gate copies
                            emit_vblk(1)
                        elif t4 == 1:
                            emit_vblk(2)
                            emit_vblk(3)

                # attention pools: 2+4+2 = 8 PSUM banks with work
                with tc.tile_pool(name="ps_s", bufs=2, space="PSUM") as ps_s, \
                     tc.tile_pool(name="ps_y", bufs=2, space="PSUM") as ps_y, \
                     tc.tile_pool(name="pp", bufs=4) as pp, \
                     tc.tile_pool(name="smalls", bufs=4) as smalls, \
                     tc.tile_pool(name="outp", bufs=8) as outp:

                    projdone = [False] * (T // 128)
                    pending_stores = []

                    def emit_proj(tcb):
                        if projdone[tcb]:
                            return False
                        projdone[tcb] = True
                        ob = outp.tile([128, C], f32, name="ob", tag="ob")
                        for oc in range(2):
                            po = work.tile([128, TQ], f32, name="po", tag="w")
                            for fcc in range(3):
                                nc.tensor.matmul(
                                    po[:, 0:CQ],
                                    lhsT=yn_sb[fcc][:, 128 * tcb:128 * (tcb + 1)],
                                    rhs=wp_sb[fcc][:, CQ * oc:CQ * (oc + 1)],
                                    start=(fcc == 0), stop=(fcc == 2),
                                )
                            nc.vector.tensor_copy(ob[:, CQ * oc:CQ * (oc + 1)], po[:, 0:CQ])
                        pending_stores.append((tcb, ob))
                        return True

                    def flush_stores(limit=None):
                        n = 0
                        while pending_stores and (limit is None or n < limit):
                            n += 1
                            tcb, ob = pending_stores.pop(0)
                            nc.sync.dma_start(
                                out=d_out.ap()[128 * tcb:128 * tcb + 64, :],
                                in_=ob[0:64, :])
                            nc.sync.dma_start(
                                out=d_out.ap()[128 * tcb + 64:128 * (tcb + 1), :],
                                in_=ob[64:128, :])

                    fillq = []

                    def fill(n=1):
                        done = 0
                        while done < n and fillq:
                            if fillq.pop(0)():
                                done += 1

                    # ---- one attention unit: head h, query chunk qi ----
                    def emit_attn_unit(h, qi, hold_stores=False):
                        if not hold_stores:
                            flush_stores(limit=2)
                        j = h // 2
                        # q/k for this unit (no-ops when already emitted)
                        emit_qkv_chain(j, qi)
                        emit_qkv_chain(3 + j, qi)
                        for kc in range(4 * qi + 4):
                            emit_vblk(kc)
                        q0 = TQ * qi
                        nkc = (q0 + TQ) // KCB
                        y = ps_y.tile([HD + 1, TQ], f32, name="y", tag="y")
                        pend = []  # attV lags one S-group behind

                        def emit_attv(p, kcs):
                            for idx, kc in enumerate(kcs):
                                r = kc - 4 * qi
                                off = KCB * r if r >= 0 else 0
                                nc.tensor.matmul(
                                    y[:, off:TQ],
                                    lhsT=v_sb[:, VSTR * kc + (HD + 1) * h:
                                              VSTR * kc + (HD + 1) * (h + 1)],
                                    rhs=p[:, TQ * idx + off:TQ * (idx + 1)],
                                    start=(kc == 0), stop=(kc == nkc - 1),
                                )

                        for g0 in range(0, nkc, SGRP):
                            kcs = list(range(g0, min(g0 + SGRP, nkc)))
                            s = ps_s.tile([128, SGRP * TQ], f32, name="s", tag="s")
                            offs = []
                            for idx, kc in enumerate(kcs):
                                r = kc - 4 * qi
                                off = KCB * r if r >= 0 else 0
                                offs.append(off)
                                nc.tensor.matmul(
                                    s[:, TQ * idx + off:TQ * (idx + 1)],
                                    lhsT=k_t[h][kc // 4][:, KCB * (kc % 4):KCB * (kc % 4 + 1)],
                                    rhs=q_t[h][qi][:, off:TQ],
                                    start=True, stop=True,
                                )
                            p = pp.tile([128, SGRP * TQ], bf16, name="p", tag="p")
                            if offs[-1] == 0:  # no trim in this group
                                w = len(kcs) * TQ
                                nc.scalar.activation(p[:, 0:w], s[:, 0:w], EXP)
                            else:  # diagonal group: exp only the live region
                                for idx in range(len(kcs)):
                                    lo, hi = TQ * idx + offs[idx], TQ * (idx + 1)
                                    nc.scalar.activation(p[:, lo:hi], s[:, lo:hi], EXP)
                            for idx, kc in enumerate(kcs):
                                r = kc - 4 * qi
                                if r >= 0:  # diagonal block: triangle mask
                                    lo = TQ * idx + KCB * r
                                    nc.gpsimd.tensor_mul(
                                        p[:, lo:lo + KCB], p[:, lo:lo + KCB], mask_sb)
                            pend.append((p, kcs))
                            if len(pend) > 1:
                                emit_attv(*pend.pop(0))
                            fill(1)
                        emit_attv(*pend.pop(0))
                        # normalize by softmax denominator (row HD of y)
                        rc = smalls.tile([HD + 1, TQ], f32, name="rc", tag="rc")
                        nc.vector.tensor_copy(rc[HD:HD + 1, :], y[HD:HD + 1, :])
                        rb = smalls.tile([HD, TQ], f32, name="rb", tag="rb")
                        lsrc = rc[HD:HD + 1, :]
                        for qd in range(4):
                            nc.sync.dma_start(
                                out=rb[16 * qd:16 * (qd + 1), :],
                                in_=bass.AP(tensor=lsrc.tensor, offset=lsrc.offset,
                                            ap=[lsrc.ap[0], [0, 16], lsrc.ap[1]]))
                        nc.vector.reciprocal_approx_fast(out=rb, in_=rb)
                        fc, half = h // 2, h % 2
                        nc.vector.tensor_mul(
                            yn_sb[fc][HD * half:HD * (half + 1), q0:q0 + TQ],
                            y[0:HD, :], rb)

                    # filler queue: pair-0 t4=3 qkv, rest of v, later pairs
                    fillq.append(lambda: emit_qkv_chain(0, 3))
                    fillq.append(lambda: emit_qkv_chain(3, 3))
                    for kc in range(4, NKC):
                        fillq.append(lambda kc=kc: emit_vblk(kc))
                    for jc in (1, 4, 2, 5):
                        for t4 in range(NQC):
                            fillq.append(lambda jc=jc, t4=t4: emit_qkv_chain(jc, t4))

                    # pairs 0 and 1: qi-outer (matches the t4-ordered copy
                    # availability at startup)
                    for pair in range(2):
                        for qi in range(NQC):
                            for h in (2 * pair, 2 * pair + 1):
                                emit_attn_unit(h, qi)
                        # next pair's q/k ready before its first unit
                        for jc in (pair + 1, pair + 4):
                            for t4 in range(NQC):
                                emit_qkv_chain(jc, t4)
                        if pair == 0:  # wp load: ring quiet, proj far away
                            for i in range(3):
                                nc.sync.dma_start(
                                    out=wp_sb[i], in_=d_wp.ap()[128 * i:128 * (i + 1), :])
                    # pair 2: qi-outer so proj column blocks release early;
                    # stores held during the last chunk so the final
                    # normalize broadcasts aren't stuck behind them.
                    for qi in range(NQC):
                        for h in (4, 5):
                            emit_attn_unit(h, qi, hold_stores=(qi == NQC - 1))
                        for tcb in range(4 * qi, 4 * qi + 4):
                            fillq.append(lambda tcb=tcb: emit_proj(tcb))
                    # drain remaining proj + stores
                    for tcb in range(T // 128):
                        emit_proj(tcb)
                        flush_stores()

    nc.compile()
    return nc


def _prep_core(x, w_attn, b_attn, w_proj, c):
    b, g = c // 2, c % 2
    h0 = NH * g
    q = slice(64 * h0, 64 * h0 + CQ)
    k = slice(C + 64 * h0, C + 64 * h0 + CQ)
    v = slice(2 * C + 64 * h0, 2 * C + 64 * h0 + CQ)

    xt = np.ascontiguousarray(x[b].T).astype(_BF16)

    wqk = np.empty((CPAD, 2 * CQ), dtype=_BF16)
    wqk[:, 0:CQ] = (w_attn[:, q] * 0.125).astype(_BF16)
    wqk[:, CQ:] = w_attn[:, k].astype(_BF16)
    bqk6 = np.concatenate([b_attn[q] * 0.125, b_attn[k]]).astype(np.float32)
    bqk6 = np.ascontiguousarray(bqk6.reshape(6, 128).T)  # [128, 6]
    # masked biases + half-masks: col 2jc (+1) = bias for even (odd) head
    # half; cols 12/13 = the 0/1 half-masks themselves.
    m0 = np.zeros((128, 1), np.float32); m0[0:64] = 1.0
    m1 = 1.0 - m0
    bqk = np.empty((128, 14), np.float32)
    for jc in range(6):
        bqk[:, 2 * jc:2 * jc + 1] = bqk6[:, jc:jc + 1] * m0
        bqk[:, 2 * jc + 1:2 * jc + 2] = bqk6[:, jc:jc + 1] * m1
    bqk[:, 12:13] = m0
    bqk[:, 13:14] = m1

    wv = np.ascontiguousarray(w_attn[:, v]).astype(_BF16)
    bv = np.ascontiguousarray(b_attn[v].reshape(1, CQ)).astype(np.float32)

    wp = np.ascontiguousarray(w_proj[q, :]).astype(_BF16)

    ii = np.arange(KCB)
    mk = (ii[:, None] <= ii[None, :]).astype(_BF16)
    return {"xt": xt, "wqk": wqk, "wv": wv, "wp": wp, "mk": mk, "bqk": bqk,
            "bv": bv}


def kernel(x, w_attn, b_attn, w_proj, b_proj):
    from concourse.bass_utils import run_bass_kernel_spmd

    x = np.asarray(x, dtype=np.float32)
    w_attn = np.asarray(w_attn, dtype=np.float32)
    b_attn = np.asarray(b_attn, dtype=np.float32)
    w_proj = np.asarray(w_proj, dtype=np.float32)
    b_proj = np.asarray(b_proj, dtype=np.float32)

    if "nc" not in _cache:
        _cache["nc"] = _build()
    nc = _cache["nc"]

    in_maps = [_prep_core(x, w_attn, b_attn, w_proj, c) for c in range(NCORES)]
    res = run_bass_kernel_spmd(nc, in_maps, core_ids=list(range(NCORES)))

    out = np.empty((B, T, C), dtype=np.float32)
    for b in range(B):
        out[b] = res.results[2 * b]["out"] + res.results[2 * b + 1]["out"] + b_proj
    return out
